# revision 74
# baseline (speedup 1.0000x reference)
"""Trainium2 Bass kernel for nn_MissTSM (B=128, W=2048, F=D=OUT=8).

Strategy
--------
Data-parallel over batch: core c handles batches [16c, 16c+16).

Algebraic collapse (emb Linear + LN1 + posenc + LN2 + 1-head single-query
attention + 2 linear projections) to a per-scalar chain; see derivation in
comments below.  Per element s = x[b,w,f]:

    y    = A(s+h0)^2 + k0          r  = y^-1/2        q = s*r
    var2 = q*Tq[w,f] + r*Tr[w,f] + (1 - eps*r^2) + T0[w,f]
    rs2  = var2^-1/2
    l    = (kq*q + kr*r + kp[w,f] - 40*m) * rs2       e = exp(l)
    gh   = e*rs2   bh = gh*r   ah = bh*s

On-chip layout: partition p = 8*cb + f (cb = local batch 0..15, f = 0..7),
free dim = w (2048).  All f-contractions (softmax denominator Z = sum_f e,
channel sums S/B1/A1, and the Hy[f,o] block contraction) are PE matmuls
with 8x8 block-diagonal / block-ones stationaries -- no transposes, no DVE
reduces.  rsqrt is exp(-0.5*ln(.)): Ln+Exp+Square live in ONE activation
table set, so the ACT engine never reloads tables mid-kernel.

The device returns the unnormalized contraction OUT_Hy[8cb+o, w] plus the
per-(b,w) scalars Z, S, B1, A1 (packed into psum rows 4cb+j by the ones-
block stationaries).  Host applies the rank-1 terms and normalization:
    out = (OUT_Hy + A1*vqo + B1*vro + S*(Hb+Hx[w])) / Z + C2
(Logit constants drop by softmax shift invariance; sum_f aw = 1 absorbs
all attention-independent biases into C2.)
"""

import numpy as np

EPS = 1e-5
B, W, NF, D, OUT = 128, 2048, 8, 8, 8
NCORES = 8
BC = B // NCORES          # 16 batches per core
P = 128                   # partitions = 8*cb + f
NCH = 4                   # w-chunks (variable widths)
CHW = [512, 512, 512, 512]
CHS = [0, 512, 1024, 1536]
WC = 512                  # max chunk width (psum tile size)
CMASK = 40.0              # mask logit offset (pre-rs2 scaling)
SCHED_RA = 0.0   # ns floor pitch for stage_a(c) (scheduler-only)
SCHED_TB = 0.0   # ns floor base for stage_b
SCHED_RB = 0.0   # ns floor pitch for stage_b(c)

_CACHE = {}
PRIO_A_STEP = -8
_LABELS = {}


def _precompute(params):
    """Host-side constant/table computation in float64."""
    w0 = np.asarray(params["emb_w"], np.float64)[:, 0]
    b0 = np.asarray(params["emb_b"], np.float64)
    g1 = np.asarray(params["emb_ln_g"], np.float64)
    bb1 = np.asarray(params["emb_ln_b"], np.float64)
    g2 = np.asarray(params["ln_g"], np.float64)
    b2 = np.asarray(params["ln_b"], np.float64)
    vq_ = np.asarray(params["var_query"], np.float64).reshape(-1)
    Win = np.asarray(params["in_proj_w"], np.float64)
    bin_ = np.asarray(params["in_proj_b"], np.float64)
    Wo = np.asarray(params["out_proj_w"], np.float64)
    bo = np.asarray(params["out_proj_b"], np.float64)
    Wp = np.asarray(params["proj_w"], np.float64)
    bp = np.asarray(params["proj_b"], np.float64)

    wc = w0 - w0.mean()
    bc = b0 - b0.mean()
    A = (wc ** 2).mean()
    Bq = 2 * (wc * bc).mean()
    C = (bc ** 2).mean()
    h0 = Bq / (2 * A)
    k0 = C + EPS - Bq ** 2 / (4 * A)
    sA = np.sqrt(A)
    b1 = sA * h0

    W1 = wc * g1
    B1v = bc * g1
    W1c = W1 - W1.mean()
    B1c = B1v - B1v.mean()
    bb1c = bb1 - bb1.mean()
    a1 = (W1c ** 2).mean()
    a12 = (W1c * B1c).mean()
    a2 = (B1c ** 2).mean()
    # LN1 var-quadratic must equal the LinearEmbed var-quadratic (holds when
    # emb_ln_g == 1, emb_ln_b == 0, as in this module's init) -- it lets the
    # whole (p2c+c2)*rho term collapse to 1 - eps*rho.
    assert abs(a1 - A) < 1e-9 and abs(2 * a12 - Bq) < 1e-9 and abs(a2 - C) < 1e-9

    c_ = 4
    inv_freq = 1.0 / (10000.0 ** (np.arange(0, c_, 2) / np.float32(c_)))
    sx = np.arange(W, dtype=np.float32)[:, None].astype(np.float64) * inv_freq
    ex = np.stack([np.sin(sx), np.cos(sx)], -1).reshape(W, -1)      # (W,4)
    sy = np.arange(NF, dtype=np.float32)[:, None].astype(np.float64) * inv_freq
    ey = np.stack([np.sin(sy), np.cos(sy)], -1).reshape(NF, -1)     # (8,4)
    mx = ex.sum(1) / D
    my = ey.sum(1) / D
    pe = np.zeros((W, NF, D))
    pe[:, :, :4] = ex[:, None, :]
    pe[:, :, 4:] = ey[None, :, :]
    Pt = bb1c[None, None, :] + pe - mx[:, None, None] - my[None, :, None]

    pw = (W1c * Pt).mean(2)           # (W,8)
    pb = (B1c * Pt).mean(2)
    p2 = (Pt ** 2).mean(2)

    Wq, Wk, Wv = Win[:D], Win[D:2 * D], Win[2 * D:]
    bq_, bk, bv = bin_[:D], bin_[D:2 * D], bin_[2 * D:]
    qv = Wq @ vq_ + bq_
    u = (Wk.T @ qv) / np.sqrt(D)
    gu = g2 * u
    kq = float(W1c @ gu)
    kr = float(B1c @ gu)
    kp = Pt @ gu                      # (W,8)

    P2m = Wp @ Wo
    V2 = P2m @ Wv
    CC = P2m @ bv + Wp @ bo + bp
    h2v = g2[None, :] * V2            # (o,d)
    vqo = h2v @ W1c
    vro = h2v @ B1c
    Hb = h2v @ bb1c
    Hs = h2v.sum(1)
    Hx = ex @ h2v[:, :4].T - mx[:, None] * Hs[None, :]   # (W,8)
    Hy = ey @ h2v[:, 4:].T - my[:, None] * Hs[None, :]   # (8,8)
    C2 = b2 @ V2.T + CC

    f16 = np.float16
    # [8cb+f, w] tables (identical for every cb and every core)
    tqf = np.ascontiguousarray(np.tile((2 * pw).T, (BC, 1))).astype(f16)
    trf = np.ascontiguousarray(np.tile((2 * pb).T, (BC, 1))).astype(f16)
    t0c = np.ascontiguousarray((p2 + EPS + 1.0).T).astype(f16)       # (8, W)

    # stationary pack [128, 1280]:
    # I | -I | REP8 | HYB | Z | S | B1 | A1 | kq*I | kr*I
    st = np.zeros((P, 10 * P), np.float32)
    st[:, 0:128] = np.eye(P)
    st[:, 128:256] = -np.eye(P)
    st[:, 1024:1152] = kq * np.eye(P)
    st[:, 1152:1280] = kr * np.eye(P)
    for cb in range(BC):
        for f in range(NF):
            st[f, 256 + 8 * cb + f] = 1.0                 # REP8 (rows 0..7)
            for o in range(NF):
                st[8 * cb + f, 384 + 8 * cb + o] = Hy[f, o]
            st[8 * cb + f, 512 + 4 * cb + 0] = 1.0        # Z   <- e
            st[8 * cb + f, 640 + 4 * cb + 1] = 1.0        # S   <- gh0
            st[8 * cb + f, 768 + 4 * cb + 2] = 1.0        # B1  <- bh0
            st[8 * cb + f, 896 + 4 * cb + 3] = 1.0        # A1  <- ah0
    statpack = st.astype(f16)

    return dict(
        sA=float(sA), b1=float(b1), k0=float(k0), kq=kq, kr=kr,
        seps=float(np.sqrt(EPS)),
        kp=kp.astype(np.float32),
        tqf=tqf, trf=trf, t0c=t0c, statpack=statpack,
        vqo=vqo.astype(np.float32), vro=vro.astype(np.float32),
        Hb=Hb.astype(np.float32), Hx=Hx.astype(np.float32),
        C2=C2.astype(np.float32),
    )


def _build_program(pre):
    import concourse.bacc as bacc
    import concourse.tile as tile
    from concourse import mybir

    dt = mybir.dt
    AF = mybir.ActivationFunctionType
    OP = mybir.AluOpType
    f16, f32 = dt.float16, dt.float32

    nc = bacc.Bacc("TRN2", target_bir_lowering=False, debug=False)

    x_d = nc.dram_tensor("xt", [P, W], f16, kind="ExternalInput")
    mkp_d = nc.dram_tensor("mkp", [P, W], f16, kind="ExternalInput")
    tq_d = nc.dram_tensor("tqf", [P, W], f16, kind="ExternalInput")
    tr_d = nc.dram_tensor("trf", [P, W], f16, kind="ExternalInput")
    t0_d = nc.dram_tensor("t0c", [NF, W], f16, kind="ExternalInput")
    st_d = nc.dram_tensor("statpack", [P, 10 * P], f16, kind="ExternalInput")
    oh_d = nc.dram_tensor("outhy", [P, W], f16, kind="ExternalOutput")
    sm_d = nc.dram_tensor("sums", [64, W], f16, kind="ExternalOutput")

    sA, b1, k0 = pre["sA"], pre["b1"], pre["k0"]
    kq, kr, seps = pre["kq"], pre["kr"], pre["seps"]

    def _lab(bi, label):
        try:
            _LABELS[bi.ins.name] = label
        except Exception:
            pass
        return bi

    with tile.TileContext(nc) as tc:
        with (
            tc.tile_pool(name="cons", bufs=1) as cons,
            tc.tile_pool(name="wk", bufs=3) as wk,
            tc.tile_pool(name="v2p", bufs=3, space="PSUM") as v2p,
            tc.tile_pool(name="smp", bufs=2, space="PSUM") as smp,
            tc.tile_pool(name="ohp", bufs=2, space="PSUM") as ohp,
        ):
            xsb = cons.tile([P, W], f16, tag="x")
            tq = cons.tile([P, W], f16, tag="tq")
            tr = cons.tile([P, W], f16, tag="tr")
            st = cons.tile([P, 10 * P], f16, tag="st")
            t0sb = cons.tile([NF, W], f16, tag="t0")
            mkp = cons.tile([P, W], f16, tag="mkp")
            nc.sync.dma_start(xsb[:, 0:512], x_d[:, 0:512])
            nc.sync.dma_start(tq[:, 0:W // 2], tq_d[:, 0:W // 2])
            nc.sync.dma_start(xsb[:, 512:1024], x_d[:, 512:1024])
            nc.sync.dma_start(tr[:, 0:W // 2], tr_d[:, 0:W // 2])
            nc.sync.dma_start(xsb[:, 1024:1536], x_d[:, 1024:1536])
            nc.sync.dma_start(xsb[:, 1536:], x_d[:, 1536:])
            nc.sync.dma_start(st[:], st_d[:])
            nc.sync.dma_start(t0sb[:], t0_d[:])
            nc.sync.dma_start(tq[:, W // 2:], tq_d[:, W // 2:])
            nc.sync.dma_start(tr[:, W // 2:], tr_d[:, W // 2:])
            nc.sync.dma_start(mkp[:], mkp_d[:])

            cb1 = cons.tile([P, 1], f32, tag="cb1")
            nc.gpsimd.memset(cb1[:], b1)
            ck0 = cons.tile([P, 1], f32, tag="ck0")
            nc.gpsimd.memset(ck0[:], k0)
            cz = cons.tile([P, 1], f32, tag="cz")
            nc.gpsimd.memset(cz[:], 0.0)

            ident = st[:, 0:128]
            nident = st[:, 128:256]
            rep8 = st[0:NF, 256:384]
            hyb = st[:, 384:512]
            onesZ = st[:, 512:576]
            onesS = st[:, 640:704]
            onesB = st[:, 768:832]
            onesA = st[:, 896:960]
            kqI = st[:, 1024:1152]
            krI = st[:, 1152:1280]

            saved = [None] * NCH

            def stage_a(c):
                cw = CHW[c]
                gs = slice(CHS[c], CHS[c] + cw)
                cs = slice(0, cw)
                yp = wk.tile([P, WC], f16, tag="yp")
                if c == 0:
                    _lab(nc.scalar.activation(yp[:, cs], xsb[:, gs], AF.Square,
                                              bias=cb1[:], scale=sA), "yp")
                else:
                    xs = wk.tile([P, WC], f16, tag="xs")
                    _lab(nc.vector.tensor_scalar(xs[:, cs], xsb[:, gs], sA, b1,
                                                 OP.mult, OP.add), "xs")
                    _lab(nc.vector.tensor_mul(yp[:, cs], xs[:, cs], xs[:, cs]), "yp")
                lny = wk.tile([P, WC], f32, tag="lny")
                _lab(nc.scalar.activation(lny[:, cs], yp[:, cs], AF.Ln, bias=ck0[:]), "lny")
                r = wk.tile([P, WC], f16, tag="r")
                _lab(nc.scalar.activation(r[:, cs], lny[:, cs], AF.Exp, bias=cz[:],
                                          scale=-0.5), "r")
                kxr = wk.tile([P, WC], f16, tag="kxr")
                _lab(nc.gpsimd.tensor_scalar(kxr[:, cs], xsb[:, gs], kq, kr,
                                             OP.mult, OP.add), "kxr")
                q = wk.tile([P, WC], f16, tag="q")
                _lab(nc.vector.tensor_mul(q[:, cs], xsb[:, gs], r[:, cs]), "q")
                p1 = wk.tile([P, WC], f16, tag="p1")
                _lab(nc.vector.tensor_mul(p1[:, cs], q[:, cs], tq[:, gs]), "p1")
                p2t = wk.tile([P, WC], f16, tag="p2")
                _lab(nc.vector.tensor_mul(p2t[:, cs], r[:, cs], tr[:, gs]), "p2")
                lr = wk.tile([P, WC], f16, tag="lr")
                _lab(nc.vector.tensor_mul(lr[:, cs], kxr[:, cs], r[:, cs]), "lr")
                l2 = wk.tile([P, WC], f16, tag="l2")
                _lab(nc.vector.tensor_add(l2[:, cs], lr[:, cs], mkp[:, gs]), "l2")
                v2 = v2p.tile([P, WC], f32, tag="v2")
                _lab(nc.tensor.matmul(v2[:, cs], ident, p1[:, cs], start=True, stop=False), "mmP1")
                _lab(nc.tensor.matmul(v2[:, cs], ident, p2t[:, cs], start=False, stop=False), "mmP2")
                _lab(nc.tensor.matmul(v2[:, cs], rep8, t0sb[:, gs], start=False, stop=True), "mmT0")
                saved[c] = (r, q, l2, v2)

            def stage_b(c):
                cw = CHW[c]
                gs = slice(CHS[c], CHS[c] + cw)
                cs = slice(0, cw)
                r, q, l2, v2 = saved[c]
                lnv = wk.tile([P, WC], f32, tag="lnv")
                _lab(nc.scalar.activation(lnv[:, cs], v2[:, cs], AF.Ln, bias=cz[:]), "lnv")
                rs2 = wk.tile([P, WC], f16, tag="rs2")
                _lab(nc.scalar.activation(rs2[:, cs], lnv[:, cs], AF.Exp, bias=cz[:],
                                          scale=-0.5), "rs2")
                l = wk.tile([P, WC], f16, tag="l")
                _lab(nc.vector.tensor_mul(l[:, cs], l2[:, cs], rs2[:, cs]), "l")
                e = wk.tile([P, WC], f16, tag="e")
                _lab(nc.scalar.activation(e[:, cs], l[:, cs], AF.Exp, bias=cz[:]), "e")
                sm = smp.tile([64, WC], f32, tag="sm")
                _lab(nc.tensor.matmul(sm[:, cs], onesZ, e[:, cs], start=True, stop=False), "mmZ")
                gh0 = wk.tile([P, WC], f16, tag="gh0")
                _lab(nc.vector.tensor_mul(gh0[:, cs], e[:, cs], rs2[:, cs]), "gh0")
                _lab(nc.tensor.matmul(sm[:, cs], onesS, gh0[:, cs], start=False, stop=False), "mmS")
                oh = ohp.tile([P, WC], f32, tag="oh")
                _lab(nc.tensor.matmul(oh[:, cs], hyb, gh0[:, cs], start=True, stop=True), "mmHy")
                ohs = wk.tile([P, WC], f16, tag="ohs")
                _lab(nc.scalar.activation(ohs[:, cs], oh[:, cs], AF.Identity,
                                          bias=cz[:]), "ohs")
                _lab(nc.sync.dma_start(oh_d[:, gs], ohs[:, cs]), "dmaOH")
                bh0 = wk.tile([P, WC], f16, tag="bh0")
                _lab(nc.vector.tensor_mul(bh0[:, cs], gh0[:, cs], r[:, cs]), "bh0")
                _lab(nc.tensor.matmul(sm[:, cs], onesB, bh0[:, cs], start=False, stop=False), "mmB")
                ah0 = wk.tile([P, WC], f16, tag="ah0")
                _lab(nc.vector.tensor_mul(ah0[:, cs], gh0[:, cs], q[:, cs]), "ah0")
                _lab(nc.tensor.matmul(sm[:, cs], onesA, ah0[:, cs], start=False, stop=True), "mmA")
                sms = wk.tile([64, WC], f16, tag="sms")
                _lab(nc.vector.tensor_copy(sms[:, cs], sm[:, cs]), "sms")
                _lab(nc.sync.dma_start(sm_d[:, gs], sms[:, cs]), "dmaSM")

            def fa(c):
                with tc.high_priority(offset=60 + c * PRIO_A_STEP):
                    stage_a(c)

            def fb(c):
                if c == NCH - 1:
                    with tc.high_priority(offset=250):
                        stage_b(c)
                else:
                    stage_b(c)

            fa(0)
            fa(1)
            fb(0)
            fa(2)
            fb(1)
            fa(3)
            fb(2)
            fb(3)

    nc.compile()
    # The act-table insertion pass greedily picks the FIRST func-set
    # containing each activation (Ln -> natural_log, Exp -> exp_and_others),
    # reloading 1.3us tables on every Ln/Exp transition.  All our
    # activations (Square/Ln/Exp/Identity) live together in the
    # natural_log_exp_and_others set: keep one load of that set, drop the
    # rest.
    from concourse.hw_specs import get_activation_tables
    tabs = get_activation_tables(nc.m.arch)
    combined_id = list(tabs).index("natural_log_exp_and_others")
    used = {i.func for b in nc.main_func.blocks for i in b.instructions
            if isinstance(i, mybir.InstLoadActFuncSet) is False
            and isinstance(i, mybir.InstActivation)
            and i.engine == mybir.EngineType.Activation}
    assert used <= tabs["natural_log_exp_and_others"], used
    first = True
    for b in nc.main_func.blocks:
        keep = []
        the_load = None
        for i in b.instructions:
            if isinstance(i, mybir.InstLoadActFuncSet):
                if first:
                    i.act_func_set_id = combined_id
                    the_load = i
                    first = False
                continue
            keep.append(i)
        if the_load is not None:
            keep.insert(0, the_load)
        b.instructions[:] = keep
    return nc


def _pack_bwf(a):
    """(BC, W, F) -> (128, W) with partition p = 8*cb + f."""
    return np.ascontiguousarray(a.transpose(0, 2, 1).reshape(P, W))


def _core_inputs(pre, x, mkp_full, core):
    return {
        "xt": _pack_bwf(x[core * BC:(core + 1) * BC]).astype(np.float16),
        "mkp": _pack_bwf(mkp_full[core * BC:(core + 1) * BC]).astype(np.float16),
        "tqf": pre["tqf"], "trf": pre["trf"], "t0c": pre["t0c"],
        "statpack": pre["statpack"],
    }


def _finalize_core(pre, outhy, sums):
    """Apply rank-1 terms + softmax normalization; returns (BC, W, OUT)."""
    oh = outhy.astype(np.float32).reshape(BC, NF, W).transpose(0, 2, 1)
    sm = sums.astype(np.float32).reshape(BC, 4, W)
    Z, S, B1s, A1s = sm[:, 0], sm[:, 1], sm[:, 2], sm[:, 3]
    num = (oh
           + A1s[:, :, None] * pre["vqo"][None, None, :]
           + B1s[:, :, None] * pre["vro"][None, None, :]
           + S[:, :, None] * (pre["Hb"][None, None, :] + pre["Hx"][None, :, :]))
    return num / Z[:, :, None] + pre["C2"][None, None, :]


def kernel(**inputs):
    from concourse.bass_utils import run_bass_kernel_spmd

    x = np.asarray(inputs["x"], np.float32)
    m = np.asarray(inputs["m"])
    params = {k: v for k, v in inputs.items() if k not in ("x", "m")}

    pre = _precompute(params)
    if "prog" not in _CACHE:
        _CACHE["prog"] = _build_program(pre)
    nc = _CACHE["prog"]

    mkp_full = pre["kp"][None] - CMASK * m.astype(np.float32)   # (B, W, 8)
    in_maps = [_core_inputs(pre, x, mkp_full, c) for c in range(NCORES)]
    res = run_bass_kernel_spmd(nc, in_maps, core_ids=list(range(NCORES)))

    out = np.empty((B, W, OUT), np.float32)
    for c in range(NCORES):
        out[c * BC:(c + 1) * BC] = _finalize_core(
            pre, res.results[c]["outhy"], res.results[c]["sums"])
    return out
